# revision 31
# baseline (speedup 1.0000x reference)
"""MaskLinear kernel for 8x TRN2 NeuronCores.

Computes out[m,d] = sum_n weight[n] * masks[m,n] * x[n,d] + bias
 (= (masks * weight) @ x + bias), with x:[100000,256], masks:[64,100000].

Strategy: shard the contraction axis N across 8 cores. Each core gets a
12500-row slice (zero-padded to 12544 = 98*128 rows = "chunks" of 128),
computes a partial [M,D] (or two column-tiled halves in bf16 mode), and
the host sums the 8 partials and adds bias.

Device per core: the host packs weight+maskT+x into ONE DRAM tensor laid
out so each ramped group of chunks is a single per-partition-contiguous
DMA on one queue (no small-transfer starvation). Per group, one
broadcast tensor_mul folds the weight column into the transposed-mask
block; the PE accumulates chunk matmuls (lhsT=[128,64] maskT*w,
rhs=[128,256] x) into PSUM. MODE selects numerics: "f16" (default)
halves traffic and runs chunk pairs concurrently via PE column tiling,
with the weight pre-scaled by 2^8 so fp16 stays in the normal range
(~4e-4 rel err, ~37us); "f32r" keeps fp32 traffic with tf32-rate
matmuls (~2e-4, ~58us); "bf16" is like f16 at ~3e-3.
"""

import numpy as np

import concourse.bacc as bacc
import concourse.mybir as mybir
from concourse import tile
from concourse.bass_utils import run_bass_kernel_spmd

N_CORES = 8
N = 100000
D = 256
M = 64
NS = N // N_CORES          # 12500 rows per shard
CHUNK = 128                # matmul contraction tile (partition dim)
C = -(-NS // CHUNK)        # 98 chunks
NP = C * CHUNK             # 12544 padded rows per shard
MW = M + 1                 # weight col + 64 mask cols
GW = MW + D                # packed row width: mask row + x row


def _groups(mode):
    """Ramped DMA group sizes (in chunks): small leading groups get the PE
    fed within ~1us of the first DMA landing; steady 14-chunk groups keep
    PE idle gaps under the ~3.4us HAM re-throttle window; small tail groups
    shorten the post-DMA critical chain. All even so chunks pair up."""
    if mode in ("bf16", "f16"):
        # PE has 2x slack at 2-byte dtypes, so no ramp-up is needed: fewer,
        # larger groups (all issued upfront, <=8 sem lanes) maximize stream
        # efficiency; small tail groups shorten the post-DMA chain.
        return [14, 14, 14, 14, 14, 14, 8, 6]
    # f32r: DMA paces the middle (~46us stream), but the last group's
    # serial chain (DMA -> mul -> B matmuls at ~414ns each) lands after the
    # final DMA - ramp the tail down so that chain is ~1us, not ~6us.
    return [2, 4, 8, 14, 14, 14, 14, 12, 8, 6, 2]


def _nwarm(mode):
    # fp32r matmuls run ~427ns regardless of the HAM clock state, and PE is
    # near-pacing there - warmup would only delay the stream.
    return 6 if mode in ("bf16", "f16") else 0


for _m in ("bf16", "f16", "f32r"):
    assert sum(_groups(_m)) == C
    assert all(g % 2 == 0 for g in _groups(_m))

MODE = "f16"               # "f16" | "f32r" | "bf16"

_STATE = {}


def _np_dtype(mode):
    if mode == "bf16":
        import ml_dtypes
        return np.dtype(ml_dtypes.bfloat16)
    if mode == "f16":
        return np.dtype(np.float16)
    return np.dtype(np.float32)


def _build_nc(mode):
    nc = bacc.Bacc("TRN2", target_bir_lowering=False, debug=False,
                   num_devices=N_CORES)

    f32 = mybir.dt.float32
    bf16 = mybir.dt.bfloat16
    if mode == "f32r":
        mm_dt = mybir.dt.float32r
        dve_view = f32     # DVE ALU ops reject f32r operands; bitcast to f32
    elif mode == "bf16":
        mm_dt = bf16
        dve_view = bf16
    elif mode == "f16":
        mm_dt = mybir.dt.float16
        dve_view = mybir.dt.float16
    else:
        raise ValueError(mode)

    # Single packed input: per group, per partition, B mask rows (MW wide)
    # then B x rows (D wide), contiguous. One DMA per group on one queue -
    # no small-packet starvation against the bulk x stream.
    # bf16 runs chunk pairs concurrently in PE col groups 0-1/2-3 into two
    # psum partition halves (folded on host); f32r matmuls use the full
    # array width internally (hi/lo column split), so they accumulate a
    # single [M, D] chain instead.
    col_tile = mode in ("bf16", "f16")
    OUTP = 2 * M if col_tile else M
    pk = nc.dram_tensor("pk", [CHUNK, C * GW], mm_dt, kind="ExternalInput")
    out = nc.dram_tensor("out", [OUTP, D], f32, kind="ExternalOutput")

    with tile.TileContext(nc) as tc:
        with (
            tc.tile_pool(name="cn", bufs=1) as cn,
            tc.tile_pool(name="gp", bufs=1) as gp,
            tc.tile_pool(name="wp", bufs=1) as wp,
            tc.tile_pool(name="pp", bufs=1, space="PSUM") as pp,
            tc.tile_pool(name="op", bufs=1) as op,
        ):
            GROUPS = _groups(mode)
            NWARM = _nwarm(mode)
            if NWARM:
                # HAM warmup: junk bf16 matmuls keep the PE array busy while
                # the leading DMAs are in flight so the clock gate opens to
                # 8/8 before (or soon after) real work arrives.
                jz = cn.tile([CHUNK, 512], bf16)
                wz = cn.tile([CHUNK, 1], bf16)
                nc.vector.memset(jz[:], 0.0)
                nc.vector.memset(wz[:], 0.0)
                pwarm = pp.tile([1, 512], f32, tag="pwarm")
                for i in range(NWARM):
                    nc.tensor.matmul(pwarm[:], wz[:], jz[:],
                                     start=(i == 0), stop=(i == NWARM - 1))

            psum = pp.tile([OUTP, D], f32)
            cbase = 0
            for g, B in enumerate(GROUPS):
                pkt = gp.tile([CHUNK, B * GW], mm_dt, tag=f"pk{g}")
                eng = nc.sync if g % 2 == 0 else nc.scalar
                eng.dma_start(pkt[:], pk[:, cbase * GW:(cbase + B) * GW])
                mt = pkt[:, :B * MW]
                xt = pkt[:, B * MW:]

                # wm[:, b, :] = mt[:, b, 1:] * mt[:, b, 0] for all b at once.
                # Inputs viewed as f32/bf16 (DVE ALU rejects f32r); the out AP
                # keeps the matmul dtype so the BIR verifier accepts f32r.
                wm = wp.tile([CHUNK, B * M], mm_dt, tag=f"wm{g}")
                mt3 = mt.bitcast(dve_view).rearrange("p (b j) -> p b j", b=B)
                wm3 = wm[:].rearrange("p (b j) -> p b j", b=B)
                nc.vector.tensor_mul(
                    wm3,
                    mt3[:, :, 1:MW],
                    mt3[:, :, 0:1].broadcast_to((CHUNK, B, M)),
                )
                if col_tile:
                    for b in range(0, B, 2):
                        cp = (cbase + b) // 2
                        # Chunk pair: col groups 0-1 and 2-3 run concurrently,
                        # accumulating into disjoint psum partition halves.
                        nc.tensor.matmul(
                            psum[0:M, :],
                            wm[:, b * M:(b + 1) * M],
                            xt[:, b * D:(b + 1) * D],
                            start=(cp == 0),
                            stop=(cp == C // 2 - 1),
                            tile_position=(0, 0),
                        )
                        nc.tensor.matmul(
                            psum[M:2 * M, :],
                            wm[:, (b + 1) * M:(b + 2) * M],
                            xt[:, (b + 1) * D:(b + 2) * D],
                            start=(cp == 0),
                            stop=(cp == C // 2 - 1),
                            tile_position=(0, M),
                        )
                else:
                    for b in range(B):
                        c = cbase + b
                        nc.tensor.matmul(
                            psum[:],
                            wm[:, b * M:(b + 1) * M],
                            xt[:, b * D:(b + 1) * D],
                            start=(c == 0),
                            stop=(c == C - 1),
                        )
                cbase += B
            osb = op.tile([OUTP, D], f32)
            nc.vector.tensor_copy(osb[:], psum[:])
            nc.sync.dma_start(out[:, :], osb[:])
    nc.compile()
    return nc


def _get_nc(mode):
    key = "nc_" + mode
    if key not in _STATE:
        _STATE[key] = _build_nc(mode)
    return _STATE[key]


def _shard_inputs(x, masks, weight, mode):
    dt = _np_dtype(mode)
    x = np.asarray(x, dtype=np.float32)
    masks = np.asarray(masks, dtype=np.float32)
    weight = np.asarray(weight, dtype=np.float32)

    in_maps = []
    for s in range(N_CORES):
        lo = s * NS
        hi = lo + NS
        xs = np.zeros((NP, D), dt)
        xs[:NS] = x[lo:hi].astype(dt, copy=False)
        ms = np.zeros((NP, MW), dt)
        # fp16: pre-scale the tiny weights (~1/sqrt(N)) by 2**8 so none land
        # in the subnormal range (exact, undone on the host after gather).
        wscale = 256.0 if mode == "f16" else 1.0
        ms[:NS, 0] = (weight[lo:hi] * wscale).astype(dt, copy=False)
        ms[:NS, 1:] = masks[:, lo:hi].T.astype(dt, copy=False)
        # Pack per group: [128, B*MW mask cols | B*D x cols], so each group
        # is one contiguous-per-partition DMA. Row (cbase*128 + p*B + b)
        # lands on partition p as sub-chunk b.
        blocks = []
        cbase = 0
        for B in _groups(mode):
            r0, r1 = cbase * CHUNK, (cbase + B) * CHUNK
            blocks.append(ms[r0:r1].reshape(CHUNK, B * MW))
            blocks.append(xs[r0:r1].reshape(CHUNK, B * D))
            cbase += B
        pk = np.concatenate(blocks, axis=1)
        assert pk.shape == (CHUNK, C * GW)
        in_maps.append({"pk": pk})
    return in_maps


def _run(x, masks, weight, bias, mode=MODE, **run_kwargs):
    in_maps = _shard_inputs(x, masks, weight, mode)
    try:
        res = run_bass_kernel_spmd(
            _get_nc(mode), in_maps, core_ids=list(range(N_CORES)), **run_kwargs
        )
    except Exception:
        # The runtime occasionally reports a transient unrecoverable-device
        # error that clears on the next execution; retry once.
        res = run_bass_kernel_spmd(
            _get_nc(mode), in_maps, core_ids=list(range(N_CORES)), **run_kwargs
        )
    parts = np.stack([r["out"] for r in res.results])  # [8, OUTP, 256]
    full = parts.sum(axis=0)
    if full.shape[0] == 2 * M:           # fold col-tiled psum halves
        full = full[:M] + full[M:]
    if mode == "f16":
        full = full * np.float32(1.0 / 256.0)
    out = full + np.asarray(bias, dtype=np.float32)
    return out.astype(np.float32), res


def kernel(x, masks, weight, bias):
    out, _ = _run(x, masks, weight, bias)
    return out
